# revision 16
# baseline (speedup 1.0000x reference)
"""GMM log-likelihood kernel for Trainium2 (Bass/Tile), 8-core data-parallel.

Math (host precompute in f64):
  B_k = L_k^{-1},  c_k = B_k mu_k,  w_k = -2 B_k^T c_k
  maha_k(x) = ||B_k x||^2 + w_k . x + ||c_k||^2
  wlp_k = -maha/2 + K_k,  K_k = log wgt_k - d/2 log 2pi - half_logdet_k
  lse(x) = m0 + log(sum_k exp(wlp_k - m0));  out = sum_x lse(x)

v5 pipeline:
  PE:   row-packed matmuls -- sample-tile A in array rows 0-63, tile B in
        rows 64-127 run CONCURRENTLY (64-deep contraction, no ones row)
        -> per 128-sample tile only ~512 streamed columns. Small 16-col
        cross matmuls (w_k . x) accumulate into a spare PSUM bank.
  drain per tile, class-mixed (period 14 = 9 A-tiles + 5 D-tiles):
    A: ACT Square (PSUM->SBUF bf16) + DVE/GPSIMD pairwise-add tree,
       levels batched per 9-tile run
    D: one DVE custom scan(ADD, sq(x)) pass -- drain + full group
       reduction; group sums = diffs of every-64th running total
  end:  maha = tree/scan sums + cross (f32 combine), batched Exp,
        x exp(K - m0 - ||c||^2/2), comp-sum, Ln(accum) -> scalar
  Host adds m0*count and subtracts the 88 zero-pad samples' lse.
"""

import numpy as np

N_COMPONENTS = 16
N_FEATURES = 64
N_SAMPLES = 200000
N_CORES = 8
PER_CORE = N_SAMPLES // N_CORES          # 25000
TILE_P = 128
N_TILES = -(-PER_CORE // TILE_P)         # 196 (ceil)
N_PAIRS = (N_TILES + 1) // 2             # 98
KD = N_COMPONENTS * N_FEATURES           # 1024

PERIOD = 14                              # class pattern period (tiles)
N_ARUN = 9                               # A-tiles per period (rest are D)
N_A = (N_TILES // PERIOD) * N_ARUN       # 126
N_D = N_TILES - N_A                      # 70
GPS_L1_NUM, GPS_L1_DEN = 6, 7            # frac of A-runs with L1 on GPSIMD
ECHUNK = 49                              # tiles per end-phase chunk
XPAIRS = 32                              # pairs per cross-PSUM bank window

_CACHE = {}


def _register_custom_ops():
    """Register the fused sq(a)+b DVE op (unused fallback) -- kept for
    compatibility; the scan op below is the production path."""
    from concourse import dve_ops
    from concourse.dve_spec import Spec, Src0, Src1, lower, sq
    from concourse.dve_uop import DveOpSpec

    name = "TENSOR_SQ_ADD_GMM"
    for op in dve_ops.OPS:
        if op.name == name:
            return op
    spec = Spec(
        body=sq(Src0) + Src1,
        reference=lambda in0, in1, s0, s1, imm2: (
            in0.astype(np.float32) ** 2 + in1.astype(np.float32)
        ),
    )
    row = dve_ops._CUSTOM_DVE_ROW_BASE + len(dve_ops.OPS)
    assert row < 0x20
    shas = {}
    for ver in ("v3", "v4"):
        tmp = DveOpSpec(name=name, opcode=row, uops=lower(spec, ver=ver),
                        rd1_en=True)
        shas[ver] = tmp.sha(ver)
    op = dve_ops.DveOp(name, spec, subdim=False, uops_sha=shas)
    dve_ops.OPS.append(op)
    dve_ops._SUB_OPCODE_FOR_NAME[name] = row
    dve_ops.CUSTOM_DVE_SPECS[name] = spec
    return op


def _register_scan_op():
    """scan(ADD, sq(Src0)): running sum of squares along the stream."""
    from concourse import dve_ops
    from concourse.dve_spec import AluOp, Spec, Src0, lower, scan, sq
    from concourse.dve_uop import DveOpSpec

    name = "TENSOR_SQ_CUMSUM_GMM"
    for op in dve_ops.OPS:
        if op.name == name:
            return op
    spec = Spec(
        body=scan(AluOp.ADD, sq(Src0)),
        reference=lambda in0, in1, s0, s1, imm2: np.cumsum(
            in0.astype(np.float32) ** 2, axis=-1),
    )
    row = dve_ops._CUSTOM_DVE_ROW_BASE + len(dve_ops.OPS)
    assert row < 0x20
    shas = {}
    for ver in ("v3", "v4"):
        tmp = DveOpSpec(name=name, opcode=row, uops=lower(spec, ver=ver),
                        rd1_en=False)
        shas[ver] = tmp.sha(ver)
    op = dve_ops.DveOp(name, spec, subdim=False, uops_sha=shas)
    dve_ops.OPS.append(op)
    dve_ops._SUB_OPCODE_FOR_NAME[name] = row
    dve_ops.CUSTOM_DVE_SPECS[name] = spec
    return op


def _is_d(t):
    return t % PERIOD >= N_ARUN


def _build_nc(n_pairs):
    import concourse.tile as tile
    from concourse import bacc, mybir

    _register_custom_ops()
    scanop = _register_scan_op()

    n_tiles = n_pairs * 2
    f32 = mybir.dt.float32
    bf16 = mybir.dt.bfloat16
    ACT = mybir.ActivationFunctionType

    nc = bacc.Bacc("TRN2", target_bir_lowering=False, debug=False,
                   num_devices=N_CORES)

    xpt = nc.dram_tensor("xpt", [n_pairs * TILE_P, TILE_P], bf16,
                         kind="ExternalInput").ap()
    bmd = nc.dram_tensor("bmd", [128, KD], bf16, kind="ExternalInput").ap()
    wtd = nc.dram_tensor("wtd", [128, N_COMPONENTS], bf16,
                         kind="ExternalInput").ap()
    vkt_d = nc.dram_tensor("vkt", [128, ECHUNK * N_COMPONENTS], f32,
                           kind="ExternalInput").ap()
    ones = nc.dram_tensor("ones", [128, 1], f32, kind="ExternalInput").ap()
    out = nc.dram_tensor("out", [1, 1], f32, kind="ExternalOutput").ap()

    K = N_COMPONENTS
    EW = ECHUNK * K                       # 784 cols per end-phase chunk
    TW = N_ARUN * K                       # 144 groups per A-run

    with tile.TileContext(nc) as tc:
        with (
            tc.tile_pool(name="const", bufs=1) as const_pool,
            tc.tile_pool(name="work", bufs=2) as work_pool,
            tc.tile_pool(name="yp", bufs=3, space="PSUM") as yp_pool,
            tc.tile_pool(name="xp", bufs=2, space="PSUM") as xp_pool,
        ):
            bm = const_pool.tile([128, KD], bf16)
            nc.scalar.dma_start(bm[:], bmd[:])
            wt8 = const_pool.tile([128, K], bf16)
            nc.scalar.dma_start(wt8[:], wtd[:])
            vkt = const_pool.tile([128, EW], f32)
            nc.scalar.dma_start(vkt[:], vkt_d[:])
            on1 = const_pool.tile([128, 1], f32)
            nc.scalar.dma_start(on1[:], ones[:])

            mbuf = const_pool.tile([128, n_tiles * K], f32)
            crsb = const_pool.tile([128, n_tiles * K], f32)
            Rbuf = const_pool.tile([128, N_D * K], f32)
            rbuf = const_pool.tile([128, n_tiles], f32)
            csum = const_pool.tile([128, 1], f32)

            a_idx = d_idx = 0
            arun = []                     # mbuf col offsets of current run
            for p in range(n_pairs):
                if p % XPAIRS == 0:
                    p0 = p
                    nx = min(XPAIRS, n_pairs - p0)
                    # concurrent row-strips must write DIFFERENT banks --
                    # one cross bank per strip
                    crossA = xp_pool.tile([128, nx * K], f32, tag="crA",
                                          bufs=1)
                    crossB = xp_pool.tile([128, nx * K], f32, tag="crB",
                                          bufs=1)
                xt2 = work_pool.tile([TILE_P, TILE_P], bf16, tag="xt2",
                                     bufs=3)
                nc.sync.dma_start(xt2[:], xpt[p * TILE_P:(p + 1) * TILE_P, :])
                lhsA = xt2[0:64, :]
                lhsB = xt2[64:128, :]
                ypA = yp_pool.tile([128, KD], f32, tag="yp")
                ypB = yp_pool.tile([128, KD], f32, tag="yp")
                # row-packed: A (rows 0-63) and B (rows 64-127) stream
                # concurrently through disjoint PE row-strips
                nc.tensor.matmul(ypA[:, 0:512], lhsA, bm[0:64, 0:512],
                                 tile_position=(0, 0))
                nc.tensor.matmul(ypB[:, 0:512], lhsB, bm[64:128, 0:512],
                                 tile_position=(64, 0))
                nc.tensor.matmul(ypA[:, 512:1024], lhsA, bm[0:64, 512:1024],
                                 tile_position=(0, 0))
                nc.tensor.matmul(ypB[:, 512:1024], lhsB,
                                 bm[64:128, 512:1024],
                                 tile_position=(64, 0))
                co = p % XPAIRS
                nc.tensor.matmul(crossA[:, co * K:(co + 1) * K], lhsA,
                                 wt8[0:64, :], tile_position=(0, 0))
                nc.tensor.matmul(crossB[:, co * K:(co + 1) * K], lhsB,
                                 wt8[64:128, :], tile_position=(64, 0))

                for h, yp in ((0, ypA), (1, ypB)):
                    t = 2 * p + h
                    if _is_d(t):
                        srun = work_pool.tile([128, KD], f32, tag="srun",
                                              bufs=2)
                        nc.vector._custom_dve(scanop, out=srun[:], in0=yp[:])
                        rv = srun[:].rearrange("p (g e) -> p g e", e=64)
                        rslot = Rbuf[:, d_idx * K:(d_idx + 1) * K].rearrange(
                            "p (g e) -> p g e", e=1)
                        nc.vector.tensor_copy(rslot, rv[:, :, 63:64])
                        d_idx += 1
                    else:
                        slot = len(arun)
                        if slot == 0:
                            sqab = work_pool.tile([128, N_ARUN * KD], bf16,
                                                  tag="sqab", bufs=2)
                        nc.scalar.activation(
                            sqab[:, slot * KD:(slot + 1) * KD], yp[:],
                            ACT.Square)
                        arun.append(t)
                        if slot == N_ARUN - 1:
                            # batched tree over the 9-tile run
                            ab = a_idx // N_ARUN
                            sv = sqab[:].rearrange("p (g e) -> p g e", e=64)
                            l1 = work_pool.tile([128, TW * 32], bf16,
                                                tag="l1", bufs=2)
                            q1 = l1[:].rearrange("p (g e) -> p g e", e=32)
                            gl1 = (ab * GPS_L1_NUM) % GPS_L1_DEN < GPS_L1_NUM
                            eng = nc.gpsimd if gl1 else nc.vector
                            h5 = 5 * K
                            eng.tensor_add(q1[:, 0:h5, :],
                                           sv[:, 0:h5, 0:32],
                                           sv[:, 0:h5, 32:64])
                            eng.tensor_add(q1[:, h5:TW, :],
                                           sv[:, h5:TW, 0:32],
                                           sv[:, h5:TW, 32:64])
                            l2 = work_pool.tile([128, TW * 16], bf16,
                                                tag="l2", bufs=2)
                            a2 = l2[:].rearrange("p (g e) -> p g e", e=16)
                            nc.vector.tensor_add(a2, q1[:, :, 0:16],
                                                 q1[:, :, 16:32])
                            l3 = work_pool.tile([128, TW * 8], bf16,
                                                tag="l3", bufs=2)
                            b3 = l3[:].rearrange("p (g e) -> p g e", e=8)
                            nc.vector.tensor_add(b3, a2[:, :, 0:8],
                                                 a2[:, :, 8:16])
                            l4 = work_pool.tile([128, TW * 4], bf16,
                                                tag="l4", bufs=2)
                            c4 = l4[:].rearrange("p (g e) -> p g e", e=4)
                            nc.vector.tensor_add(c4, b3[:, :, 0:4],
                                                 b3[:, :, 4:8])
                            l5 = work_pool.tile([128, TW * 2], bf16,
                                                tag="l5", bufs=2)
                            d5 = l5[:].rearrange("p (g e) -> p g e", e=2)
                            nc.vector.tensor_add(d5, c4[:, :, 0:2],
                                                 c4[:, :, 2:4])
                            # run tiles are consecutive: cols contiguous
                            mofs = arun[0] * K
                            mv = mbuf[:, mofs:mofs + TW].rearrange(
                                "p (g e) -> p g e", e=1)
                            nc.vector.tensor_add(mv, d5[:, :, 0:1],
                                                 d5[:, :, 1:2])
                            arun = []
                        a_idx += 1

                if p % XPAIRS == XPAIRS - 1 or p == n_pairs - 1:
                    # drain the cross banks (ACT; PSUM -> SBUF f32); even
                    # tiles came from strip A, odd from strip B
                    wv = crsb[:, 2 * p0 * K:2 * (p0 + nx) * K].rearrange(
                        "p (q c) -> p q c", c=2 * K)
                    nc.scalar.copy(wv[:, :, 0:K],
                                   crossA[:].rearrange(
                                       "p (q c) -> p q c", c=K))
                    nc.scalar.copy(wv[:, :, K:2 * K],
                                   crossB[:].rearrange(
                                       "p (q c) -> p q c", c=K))

            # batched D-tile diffs: maha_g = R_g - R_{g-1} within each tile
            nrun = n_tiles // PERIOD
            nd_run = PERIOD - N_ARUN
            for j in range(nrun):
                dlo = j * nd_run
                rall = Rbuf[:, dlo * K:(dlo + nd_run) * K].rearrange(
                    "p (d g) -> p d g", g=K)
                tlo = j * PERIOD + N_ARUN
                mdv = mbuf[:, tlo * K:(tlo + nd_run) * K].rearrange(
                    "p (d g) -> p d g", g=K)
                nc.vector.tensor_copy(mdv[:, :, 0:1], rall[:, :, 0:1])
                nc.vector.tensor_sub(mdv[:, :, 1:K], rall[:, :, 1:K],
                                     rall[:, :, 0:K - 1])

            # end phase: maha = sqsum + cross; exp; weights; lse tail
            for c in range(n_tiles // ECHUNK):
                ofs = c * EW
                cmb = work_pool.tile([128, EW], f32, tag="cmb", bufs=2)
                nc.vector.tensor_add(cmb[:], mbuf[:, ofs:ofs + EW],
                                     crsb[:, ofs:ofs + EW])
                wt = work_pool.tile([128, EW], bf16, tag="wt", bufs=2)
                nc.scalar.activation(wt[:], cmb[:], ACT.Exp, scale=-0.5)
                wf = work_pool.tile([128, EW], bf16, tag="wf", bufs=2)
                nc.vector.tensor_mul(wf[:], wt[:], vkt[:])
                ebs = work_pool.tile([128, EW // 2], bf16, tag="ebs", bufs=2)
                wv = wf[:].rearrange("p (t k) -> p t k", k=K)
                ev = ebs[:].rearrange("p (t k) -> p t k", k=K // 2)
                nc.gpsimd.tensor_add(ev, wv[:, :, 0:K // 2],
                                     wv[:, :, K // 2:K])
                nc.vector.reduce_sum(rbuf[:, c * ECHUNK:(c + 1) * ECHUNK],
                                     ev, axis=mybir.AxisListType.X)
            lnr = const_pool.tile([128, n_tiles], f32)
            nc.scalar.activation(lnr[:], rbuf[:], ACT.Ln, accum_out=csum[:])

            rp = xp_pool.tile([1, 1], f32, tag="crA", bufs=1)
            nc.tensor.matmul(rp[:], on1[:], csum[:])
            res = const_pool.tile([1, 1], f32)
            nc.scalar.copy(res[:], rp[:])
            nc.sync.dma_start(out[:], res[:])

    nc.compile()
    return nc


def _precompute(weights, means, covariances):
    """Host-side O(K d^3) prep in float64."""
    import ml_dtypes

    K, d = means.shape
    L = np.linalg.cholesky(covariances.astype(np.float64))
    half_logdet = np.log(np.diagonal(L, axis1=-2, axis2=-1)).sum(-1)
    eye = np.eye(d)
    B = np.stack([np.linalg.solve(L[k], eye) for k in range(K)])  # L^-1
    mu = means.astype(np.float64)
    c = np.einsum('kij,kj->ki', B, mu)                            # B mu
    w = -2.0 * np.einsum('kij,ki->kj', B, c)                      # -2 B^T c
    c2 = (c * c).sum(-1)                                          # ||c||^2
    Kconst = (np.log(weights.astype(np.float64))
              - 0.5 * d * np.log(2.0 * np.pi) - half_logdet)
    m0 = float(Kconst.max()) - 20.0
    v = np.exp(Kconst - m0 - 0.5 * c2)                            # [K]

    bmd = np.zeros((128, KD), np.float32)
    wtd = np.zeros((128, N_COMPONENTS), np.float32)
    for k in range(K):
        bmd[0:d, k * d:(k + 1) * d] = B[k].T.astype(np.float32)
        bmd[64:64 + d, k * d:(k + 1) * d] = B[k].T.astype(np.float32)
        wtd[0:d, k] = w[k].astype(np.float32)
        wtd[64:64 + d, k] = w[k].astype(np.float32)
    bmd = bmd.astype(ml_dtypes.bfloat16)
    wtd = wtd.astype(ml_dtypes.bfloat16)
    vkt = np.tile(v.astype(np.float32), ECHUNK)                   # [784]
    vkt = np.broadcast_to(vkt, (128, ECHUNK * N_COMPONENTS)).copy()

    # zero-pad samples: sqsum = 0, cross = 0 -> lse0 from vkt exactly
    lse0 = m0 + float(np.log(np.exp(Kconst - m0 - 0.5 * c2).sum()))
    return bmd, wtd, vkt, m0, lse0


def _make_inputs(data, bmd, wtd, vkt, n_tiles):
    """Build the 8 per-core input maps: [128, 128] pair blocks with
    sample-tile A transposed into rows 0-63, tile B into rows 64-127."""
    import ml_dtypes

    ones = np.ones((128, 1), np.float32)

    padded = n_tiles * TILE_P
    in_maps = []
    for cidx in range(N_CORES):
        sl = np.asarray(data[cidx * PER_CORE:(cidx + 1) * PER_CORE],
                        np.float32)
        xp = np.zeros((padded, N_FEATURES), np.float32)
        xp[:sl.shape[0]] = sl
        n_pairs = n_tiles // 2
        # [n_pairs, 2, 128, 64] -> [n_pairs, 2*64, 128]
        xt = xp.reshape(n_pairs, 2, TILE_P, N_FEATURES).transpose(0, 1, 3, 2)
        xpt = xt.reshape(n_pairs * TILE_P, TILE_P).astype(ml_dtypes.bfloat16)
        in_maps.append({"xpt": xpt, "bmd": bmd, "wtd": wtd, "vkt": vkt,
                        "ones": ones})
    return in_maps


def _run(data, weights, means, covariances, trace=False):
    from concourse.bass_utils import run_bass_kernel_spmd

    bmd, wtd, vkt, m0, lse0 = _precompute(np.asarray(weights),
                                          np.asarray(means),
                                          np.asarray(covariances))
    if "nc" not in _CACHE:
        _CACHE["nc"] = _build_nc(N_PAIRS)
    nc = _CACHE["nc"]

    in_maps = _make_inputs(data, bmd, wtd, vkt, N_TILES)
    res = run_bass_kernel_spmd(nc, in_maps, list(range(N_CORES)), trace=trace)
    n_pad = N_TILES * TILE_P - PER_CORE                           # 88
    total = 0.0
    for cidx in range(N_CORES):
        total += (float(res.results[cidx]["out"][0, 0])
                  + PER_CORE * m0 - n_pad * (lse0 - m0))
    return np.float32(total), res


def kernel(data, weights, means, covariances):
    return _run(data, weights, means, covariances)[0]


# revision 19
# speedup vs baseline: 1.2495x; 1.2495x over previous
"""GMM log-likelihood kernel for Trainium2 (Bass/Tile), 8-core data-parallel.

Math (host precompute in f64):
  B_k = L_k^{-1},  c_k = B_k mu_k,  w_k = -2 B_k^T c_k
  maha_k(x) = ||B_k x||^2 + w_k . x + ||c_k||^2
  wlp_k = -maha/2 + K_k,  K_k = log wgt_k - d/2 log 2pi - half_logdet_k
  lse(x) = m0 + log(sum_k exp(wlp_k - m0));  out = sum_x lse(x)

v5 pipeline:
  PE:   row-packed matmuls -- sample-tile A in array rows 0-63, tile B in
        rows 64-127 run CONCURRENTLY (64-deep contraction, no ones row)
        -> per 128-sample tile only ~512 streamed columns. Small 16-col
        cross matmuls (w_k . x) accumulate into a spare PSUM bank.
  drain per tile, class-mixed (period 14 = 9 A-tiles + 5 D-tiles):
    A: ACT Square (PSUM->SBUF bf16) + DVE/GPSIMD pairwise-add tree,
       levels batched per 9-tile run
    D: one DVE custom scan(ADD, sq(x)) pass -- drain + full group
       reduction; group sums = diffs of every-64th running total
  end:  maha = tree/scan sums + cross (f32 combine), batched Exp,
        x exp(K - m0 - ||c||^2/2), comp-sum, Ln(accum) -> scalar
  Host adds m0*count and subtracts the 88 zero-pad samples' lse.
"""

import numpy as np

N_COMPONENTS = 16
N_FEATURES = 64
N_SAMPLES = 200000
N_CORES = 8
PER_CORE = N_SAMPLES // N_CORES          # 25000
TILE_P = 128
N_TILES = -(-PER_CORE // TILE_P)         # 196 (ceil)
N_PAIRS = (N_TILES + 1) // 2             # 98
KD = N_COMPONENTS * N_FEATURES           # 1024

PERIOD = 3                               # class pattern [A, A, D]
N_D = N_TILES // PERIOD                  # 65
N_A = N_TILES - N_D                      # 131
BPER = 6                                 # periods per A-tree batch
GPS_L1 = 4                               # of 6 two-tile L1 chunks on GPSIMD
ECHUNK = 49                              # tiles per end-phase chunk
XPAIRS = 32                              # pairs per cross-PSUM bank window

_CACHE = {}


def _register_custom_ops():
    """Register the fused sq(a)+b DVE op (unused fallback) -- kept for
    compatibility; the scan op below is the production path."""
    from concourse import dve_ops
    from concourse.dve_spec import Spec, Src0, Src1, lower, sq
    from concourse.dve_uop import DveOpSpec

    name = "TENSOR_SQ_ADD_GMM"
    for op in dve_ops.OPS:
        if op.name == name:
            return op
    spec = Spec(
        body=sq(Src0) + Src1,
        reference=lambda in0, in1, s0, s1, imm2: (
            in0.astype(np.float32) ** 2 + in1.astype(np.float32)
        ),
    )
    row = dve_ops._CUSTOM_DVE_ROW_BASE + len(dve_ops.OPS)
    assert row < 0x20
    shas = {}
    for ver in ("v3", "v4"):
        tmp = DveOpSpec(name=name, opcode=row, uops=lower(spec, ver=ver),
                        rd1_en=True)
        shas[ver] = tmp.sha(ver)
    op = dve_ops.DveOp(name, spec, subdim=False, uops_sha=shas)
    dve_ops.OPS.append(op)
    dve_ops._SUB_OPCODE_FOR_NAME[name] = row
    dve_ops.CUSTOM_DVE_SPECS[name] = spec
    return op


def _register_scan_op():
    """scan(ADD, sq(Src0)): running sum of squares along the stream."""
    from concourse import dve_ops
    from concourse.dve_spec import AluOp, Spec, Src0, lower, scan, sq
    from concourse.dve_uop import DveOpSpec

    name = "TENSOR_SQ_CUMSUM_GMM"
    for op in dve_ops.OPS:
        if op.name == name:
            return op
    spec = Spec(
        body=scan(AluOp.ADD, sq(Src0)),
        reference=lambda in0, in1, s0, s1, imm2: np.cumsum(
            in0.astype(np.float32) ** 2, axis=-1),
    )
    row = dve_ops._CUSTOM_DVE_ROW_BASE + len(dve_ops.OPS)
    assert row < 0x20
    shas = {}
    for ver in ("v3", "v4"):
        tmp = DveOpSpec(name=name, opcode=row, uops=lower(spec, ver=ver),
                        rd1_en=False)
        shas[ver] = tmp.sha(ver)
    op = dve_ops.DveOp(name, spec, subdim=False, uops_sha=shas)
    dve_ops.OPS.append(op)
    dve_ops._SUB_OPCODE_FOR_NAME[name] = row
    dve_ops.CUSTOM_DVE_SPECS[name] = spec
    return op


def _is_d(t):
    return t % PERIOD == PERIOD - 1


def _build_nc(n_pairs):
    import concourse.tile as tile
    from concourse import bacc, mybir

    _register_custom_ops()
    scanop = _register_scan_op()

    n_tiles = n_pairs * 2
    f32 = mybir.dt.float32
    bf16 = mybir.dt.bfloat16
    ACT = mybir.ActivationFunctionType

    nc = bacc.Bacc("TRN2", target_bir_lowering=False, debug=False,
                   num_devices=N_CORES)

    xpt = nc.dram_tensor("xpt", [n_pairs * TILE_P, TILE_P], bf16,
                         kind="ExternalInput").ap()
    bmd = nc.dram_tensor("bmd", [128, KD], bf16, kind="ExternalInput").ap()
    wtd = nc.dram_tensor("wtd", [128, N_COMPONENTS], bf16,
                         kind="ExternalInput").ap()
    vkt_d = nc.dram_tensor("vkt", [128, ECHUNK * N_COMPONENTS], f32,
                           kind="ExternalInput").ap()
    ones = nc.dram_tensor("ones", [128, 1], f32, kind="ExternalInput").ap()
    out = nc.dram_tensor("out", [1, 1], f32, kind="ExternalOutput").ap()

    K = N_COMPONENTS
    EW = ECHUNK * K                       # 784 cols per end-phase chunk
    NAB = 2 * BPER                        # A-tiles per full tree batch

    with tile.TileContext(nc) as tc:
        with (
            tc.tile_pool(name="const", bufs=1) as const_pool,
            tc.tile_pool(name="work", bufs=2) as work_pool,
            tc.tile_pool(name="yp", bufs=3, space="PSUM") as yp_pool,
            tc.tile_pool(name="xp", bufs=2, space="PSUM") as xp_pool,
        ):
            bm = const_pool.tile([128, KD], bf16)
            nc.scalar.dma_start(bm[:], bmd[:])
            wt8 = const_pool.tile([128, K], bf16)
            nc.scalar.dma_start(wt8[:], wtd[:])
            vkt = const_pool.tile([128, EW], f32)
            nc.scalar.dma_start(vkt[:], vkt_d[:])
            on1 = const_pool.tile([128, 1], f32)
            nc.scalar.dma_start(on1[:], ones[:])

            mbuf = const_pool.tile([128, n_tiles * K], f32)
            crsb = const_pool.tile([128, n_tiles * K], f32)
            Rbuf = const_pool.tile([128, N_D * K], f32)
            rbuf = const_pool.tile([128, n_tiles], f32)
            csum = const_pool.tile([128, 1], f32)

            a_idx = d_idx = 0
            arun = []                     # mbuf col offsets of current run
            for p in range(n_pairs):
                if p % XPAIRS == 0:
                    p0 = p
                    nx = min(XPAIRS, n_pairs - p0)
                    # concurrent row-strips must write DIFFERENT banks --
                    # one cross bank per strip
                    crossA = xp_pool.tile([128, nx * K], f32, tag="crA",
                                          bufs=1)
                    crossB = xp_pool.tile([128, nx * K], f32, tag="crB",
                                          bufs=1)
                xt2 = work_pool.tile([TILE_P, TILE_P], bf16, tag="xt2",
                                     bufs=3)
                nc.sync.dma_start(xt2[:], xpt[p * TILE_P:(p + 1) * TILE_P, :])
                lhsA = xt2[0:64, :]
                lhsB = xt2[64:128, :]
                ypA = yp_pool.tile([128, KD], f32, tag="yp")
                ypB = yp_pool.tile([128, KD], f32, tag="yp")
                # row-packed: A (rows 0-63) and B (rows 64-127) stream
                # concurrently through disjoint PE row-strips
                nc.tensor.matmul(ypA[:, 0:512], lhsA, bm[0:64, 0:512],
                                 tile_position=(0, 0))
                nc.tensor.matmul(ypB[:, 0:512], lhsB, bm[64:128, 0:512],
                                 tile_position=(64, 0))
                nc.tensor.matmul(ypA[:, 512:1024], lhsA, bm[0:64, 512:1024],
                                 tile_position=(0, 0))
                nc.tensor.matmul(ypB[:, 512:1024], lhsB,
                                 bm[64:128, 512:1024],
                                 tile_position=(64, 0))
                co = p % XPAIRS
                nc.tensor.matmul(crossA[:, co * K:(co + 1) * K], lhsA,
                                 wt8[0:64, :], tile_position=(0, 0))
                nc.tensor.matmul(crossB[:, co * K:(co + 1) * K], lhsB,
                                 wt8[64:128, :], tile_position=(64, 0))

                for h, yp in ((0, ypA), (1, ypB)):
                    t = 2 * p + h
                    if _is_d(t):
                        srun = work_pool.tile([128, KD], f32, tag="srun",
                                              bufs=2)
                        nc.vector._custom_dve(scanop, out=srun[:], in0=yp[:])
                        rv = srun[:].rearrange("p (g e) -> p g e", e=64)
                        rslot = Rbuf[:, d_idx * K:(d_idx + 1) * K].rearrange(
                            "p (g e) -> p g e", e=1)
                        nc.vector.tensor_copy(rslot, rv[:, :, 63:64])
                        d_idx += 1
                    else:
                        slot = a_idx % NAB
                        if slot == 0:
                            nab = min(NAB, N_A - a_idx)
                            t0b = t          # first tile of this batch
                            sqab = work_pool.tile([128, NAB * KD], bf16,
                                                  tag="sqab", bufs=2)
                        nc.scalar.activation(
                            sqab[:, slot * KD:(slot + 1) * KD], yp[:],
                            ACT.Square)
                        if slot == nab - 1:
                            # batched tree over the 12 A-tiles; L1 in
                            # 2-tile chunks split DVE/GPSIMD
                            TW = nab * K
                            sv = sqab[:].rearrange("p (g e) -> p g e", e=64)
                            l1 = work_pool.tile([128, TW * 32], bf16,
                                                tag="l1", bufs=2)
                            q1 = l1[:].rearrange("p (g e) -> p g e", e=32)
                            nch = (nab + 1) // 2
                            for cchunk in range(nch):
                                glo = cchunk * 2 * K
                                ghi = min(glo + 2 * K, TW)
                                eng = (nc.gpsimd if cchunk < GPS_L1
                                       else nc.vector)
                                eng.tensor_add(q1[:, glo:ghi, :],
                                               sv[:, glo:ghi, 0:32],
                                               sv[:, glo:ghi, 32:64])
                            l2 = work_pool.tile([128, TW * 16], bf16,
                                                tag="l2", bufs=2)
                            a2 = l2[:].rearrange("p (g e) -> p g e", e=16)
                            nc.vector.tensor_add(a2, q1[:, :, 0:16],
                                                 q1[:, :, 16:32])
                            l3 = work_pool.tile([128, TW * 8], bf16,
                                                tag="l3", bufs=2)
                            b3 = l3[:].rearrange("p (g e) -> p g e", e=8)
                            nc.vector.tensor_add(b3, a2[:, :, 0:8],
                                                 a2[:, :, 8:16])
                            l4 = work_pool.tile([128, TW * 4], bf16,
                                                tag="l4", bufs=2)
                            c4 = l4[:].rearrange("p (g e) -> p g e", e=4)
                            nc.vector.tensor_add(c4, b3[:, :, 0:4],
                                                 b3[:, :, 4:8])
                            l5 = work_pool.tile([128, TW * 2], bf16,
                                                tag="l5", bufs=2)
                            d5 = l5[:].rearrange("p (g e) -> p g e", e=2)
                            nc.vector.tensor_add(d5, c4[:, :, 0:2],
                                                 c4[:, :, 2:4])
                            dd = work_pool.tile([128, TW], f32, tag="dd",
                                                bufs=2)
                            dv = l5[:].rearrange("p (g e) -> p g e", e=2)
                            ddv = dd[:].rearrange("p (g e) -> p g e", e=1)
                            nc.vector.tensor_add(ddv, dv[:, :, 0:1],
                                                 dv[:, :, 1:2])
                            # scatter batch sums into tile-ordered mbuf:
                            # A-tiles sit at positions {0,1} of each period
                            ntl = min(3 * ((nab + 1) // 2),
                                      n_tiles - t0b)
                            mb = mbuf[:, t0b * K:(t0b + ntl) * K].rearrange(
                                "p (a c) -> p a c", c=K)
                            d2 = dd[:].rearrange("p (a c) -> p a c", c=K)
                            nc.vector.tensor_copy(mb[:, 0::3, :],
                                                  d2[:, 0::2, :])
                            if nab > 1:
                                nc.vector.tensor_copy(mb[:, 1::3, :],
                                                      d2[:, 1::2, :])
                        a_idx += 1

                if p % XPAIRS == XPAIRS - 1 or p == n_pairs - 1:
                    # drain the cross banks (ACT; PSUM -> SBUF f32); even
                    # tiles came from strip A, odd from strip B
                    wv = crsb[:, 2 * p0 * K:2 * (p0 + nx) * K].rearrange(
                        "p (q c) -> p q c", c=2 * K)
                    nc.scalar.copy(wv[:, :, 0:K],
                                   crossA[:].rearrange(
                                       "p (q c) -> p q c", c=K))
                    nc.scalar.copy(wv[:, :, K:2 * K],
                                   crossB[:].rearrange(
                                       "p (q c) -> p q c", c=K))

            # batched D-tile diffs: maha_g = R_g - R_{g-1} within each
            # tile; D-tiles sit at position 2 of each period (stride 3)
            rall = Rbuf[:].rearrange("p (d g) -> p d g", g=K)
            mall = mbuf[:].rearrange("p (a c) -> p a c", c=K)
            mdv = mall[:, 2::3, :]
            nc.vector.tensor_copy(mdv[:, :, 0:1], rall[:, :, 0:1])
            nc.vector.tensor_sub(mdv[:, :, 1:K], rall[:, :, 1:K],
                                 rall[:, :, 0:K - 1])

            # end phase: maha = sqsum + cross; exp; weights; lse tail
            for c in range(n_tiles // ECHUNK):
                ofs = c * EW
                cmb = work_pool.tile([128, EW], f32, tag="cmb", bufs=2)
                nc.vector.tensor_add(cmb[:], mbuf[:, ofs:ofs + EW],
                                     crsb[:, ofs:ofs + EW])
                wt = work_pool.tile([128, EW], bf16, tag="wt", bufs=2)
                nc.scalar.activation(wt[:], cmb[:], ACT.Exp, scale=-0.5)
                wf = work_pool.tile([128, EW], bf16, tag="wf", bufs=2)
                nc.vector.tensor_mul(wf[:], wt[:], vkt[:])
                ebs = work_pool.tile([128, EW // 2], bf16, tag="ebs", bufs=2)
                wv = wf[:].rearrange("p (t k) -> p t k", k=K)
                ev = ebs[:].rearrange("p (t k) -> p t k", k=K // 2)
                nc.gpsimd.tensor_add(ev, wv[:, :, 0:K // 2],
                                     wv[:, :, K // 2:K])
                nc.vector.reduce_sum(rbuf[:, c * ECHUNK:(c + 1) * ECHUNK],
                                     ev, axis=mybir.AxisListType.X)
            lnr = const_pool.tile([128, n_tiles], f32)
            nc.scalar.activation(lnr[:], rbuf[:], ACT.Ln, accum_out=csum[:])

            rp = xp_pool.tile([1, 1], f32, tag="crA", bufs=1)
            nc.tensor.matmul(rp[:], on1[:], csum[:])
            res = const_pool.tile([1, 1], f32)
            nc.scalar.copy(res[:], rp[:])
            nc.sync.dma_start(out[:], res[:])

    nc.compile()
    return nc


def _precompute(weights, means, covariances):
    """Host-side O(K d^3) prep in float64."""
    import ml_dtypes

    K, d = means.shape
    L = np.linalg.cholesky(covariances.astype(np.float64))
    half_logdet = np.log(np.diagonal(L, axis1=-2, axis2=-1)).sum(-1)
    eye = np.eye(d)
    B = np.stack([np.linalg.solve(L[k], eye) for k in range(K)])  # L^-1
    mu = means.astype(np.float64)
    c = np.einsum('kij,kj->ki', B, mu)                            # B mu
    w = -2.0 * np.einsum('kij,ki->kj', B, c)                      # -2 B^T c
    c2 = (c * c).sum(-1)                                          # ||c||^2
    Kconst = (np.log(weights.astype(np.float64))
              - 0.5 * d * np.log(2.0 * np.pi) - half_logdet)
    m0 = float(Kconst.max()) - 20.0
    v = np.exp(Kconst - m0 - 0.5 * c2)                            # [K]

    bmd = np.zeros((128, KD), np.float32)
    wtd = np.zeros((128, N_COMPONENTS), np.float32)
    for k in range(K):
        bmd[0:d, k * d:(k + 1) * d] = B[k].T.astype(np.float32)
        bmd[64:64 + d, k * d:(k + 1) * d] = B[k].T.astype(np.float32)
        wtd[0:d, k] = w[k].astype(np.float32)
        wtd[64:64 + d, k] = w[k].astype(np.float32)
    bmd = bmd.astype(ml_dtypes.bfloat16)
    wtd = wtd.astype(ml_dtypes.bfloat16)
    vkt = np.tile(v.astype(np.float32), ECHUNK)                   # [784]
    vkt = np.broadcast_to(vkt, (128, ECHUNK * N_COMPONENTS)).copy()

    # zero-pad samples: sqsum = 0, cross = 0 -> lse0 from vkt exactly
    lse0 = m0 + float(np.log(np.exp(Kconst - m0 - 0.5 * c2).sum()))
    return bmd, wtd, vkt, m0, lse0


def _make_inputs(data, bmd, wtd, vkt, n_tiles):
    """Build the 8 per-core input maps: [128, 128] pair blocks with
    sample-tile A transposed into rows 0-63, tile B into rows 64-127."""
    import ml_dtypes

    ones = np.ones((128, 1), np.float32)

    padded = n_tiles * TILE_P
    in_maps = []
    for cidx in range(N_CORES):
        sl = np.asarray(data[cidx * PER_CORE:(cidx + 1) * PER_CORE],
                        np.float32)
        xp = np.zeros((padded, N_FEATURES), np.float32)
        xp[:sl.shape[0]] = sl
        n_pairs = n_tiles // 2
        # [n_pairs, 2, 128, 64] -> [n_pairs, 2*64, 128]
        xt = xp.reshape(n_pairs, 2, TILE_P, N_FEATURES).transpose(0, 1, 3, 2)
        xpt = xt.reshape(n_pairs * TILE_P, TILE_P).astype(ml_dtypes.bfloat16)
        in_maps.append({"xpt": xpt, "bmd": bmd, "wtd": wtd, "vkt": vkt,
                        "ones": ones})
    return in_maps


def _run(data, weights, means, covariances, trace=False):
    from concourse.bass_utils import run_bass_kernel_spmd

    bmd, wtd, vkt, m0, lse0 = _precompute(np.asarray(weights),
                                          np.asarray(means),
                                          np.asarray(covariances))
    if "nc" not in _CACHE:
        _CACHE["nc"] = _build_nc(N_PAIRS)
    nc = _CACHE["nc"]

    in_maps = _make_inputs(data, bmd, wtd, vkt, N_TILES)
    res = run_bass_kernel_spmd(nc, in_maps, list(range(N_CORES)), trace=trace)
    n_pad = N_TILES * TILE_P - PER_CORE                           # 88
    total = 0.0
    for cidx in range(N_CORES):
        total += (float(res.results[cidx]["out"][0, 0])
                  + PER_CORE * m0 - n_pad * (lse0 - m0))
    return np.float32(total), res


def kernel(data, weights, means, covariances):
    return _run(data, weights, means, covariances)[0]
